# revision 51
# baseline (speedup 1.0000x reference)
"""Bass/Tile MHA kernel for trn2, sharded over 8 cores as (batch, head-group).

Each core handles one batch b and 3 heads. Inputs are host-prepared bf16:
  qt, kt, vt : [D, S] bf16   — Q[b].T etc. (transposed + cast on host)
  mt         : [S, S] bf16   — mask[b,0].T  (mt[k, q] = mask[b,0,q,k]), 0/1
  wqt, wkt, wvt : [D, 3*DK] bf16 — W_X.T[:, head_cols]
  wot        : [3*DK, D] bf16    — W_O.T[head_rows, :]
Output:
  out : [S, D] fp32 — partial output (sum over the 4 head-groups of a batch
        gives the final output rows for that batch).

Structure per core:
  phase 1: load raw k, v, q (bf16, windowed DMAs) + project to
           kT/qT ([dk, S] per head, bf16) and v_sb ([s, dk+1] tiles with a
           ones column for the softmax denominator).
  phase 2: 2 q-blocks of 1024. Per block: pass A (heads 0,1) and pass B
           (head 2) over 16 k-tiles; scores accumulate into [128, 1024]
           PSUM tiles so exp runs 1024 wide on ACT (the bottleneck
           engine). Softmax denominator comes from the ones column of v.
           Output projection for block i is interleaved into block i+1's
           pass A, sharing the scores PSUM ring (tag "ps") and attn PSUM
           ring (tag "av") to stay within the 8 PSUM banks.
"""

import numpy as np

import concourse.bass as bass
import concourse.bacc as bacc
from concourse import library_config
import concourse.tile as tile
import concourse.mybir as mybir

F32 = mybir.dt.float32
BF16 = mybir.dt.bfloat16
AF = mybir.ActivationFunctionType
ALU = mybir.AluOpType

D = 768
DK = 64
NH = 3          # heads per core
HD = NH * DK    # 192
S = 2048
ST = S // 128   # 16 k-tiles
KT6 = D // 128  # 6 contraction tiles for projections
QB = 1024       # q-block width
NQ = S // QB    # 2 q-blocks


def build_mha_nc(n_cores=8, reps=1):
    nc = bacc.Bacc("TRN2", target_bir_lowering=False, debug=False,
                   num_devices=n_cores)

    qt_d = nc.dram_tensor("qt", [D, S], BF16, kind="ExternalInput")
    kt_d = nc.dram_tensor("kt", [D, S], BF16, kind="ExternalInput")
    vt_d = nc.dram_tensor("vt", [D, S], BF16, kind="ExternalInput")
    mt_d = nc.dram_tensor("mt", [S, S], BF16, kind="ExternalInput")
    wqt_d = nc.dram_tensor("wqt", [D, HD], BF16, kind="ExternalInput")
    wkt_d = nc.dram_tensor("wkt", [D, HD], BF16, kind="ExternalInput")
    wvt_d = nc.dram_tensor("wvt", [D, HD], BF16, kind="ExternalInput")
    wot_d = nc.dram_tensor("wot", [HD, D], BF16, kind="ExternalInput")
    out_d = nc.dram_tensor("out", [S, D], BF16, kind="ExternalOutput")
    qt_r = qt_d.ap().rearrange("(o p) s -> p o s", p=128)
    kt_r = kt_d.ap().rearrange("(o p) s -> p o s", p=128)
    vt_r = vt_d.ap().rearrange("(o p) s -> p o s", p=128)
    mt_r = mt_d.ap().rearrange("(o p) q -> p o q", p=128)
    out_r = out_d.ap().rearrange("(o p) d -> p o d", p=128)

    with tile.TileContext(nc) as tc:
      for _rep in range(reps):
        with (
            tc.tile_pool(name="perm", bufs=1) as perm,
            tc.tile_pool(name="mask", bufs=1) as mask_pool,
        ):
            # ---- persistent SBUF tensors ----
            qT_a = perm.tile([128, S], BF16, tag="qT_a")   # h0 rows 0-63, h1 64-127
            qT_b = perm.tile([64, S], BF16, tag="qT_b")    # h2
            kT_a = perm.tile([128, S], BF16, tag="kT_a")
            kT_b = perm.tile([64, S], BF16, tag="kT_b")
            v_sb = perm.tile([128, ST, NH, DK + 1], BF16, tag="v_sb")
            attnT_a = perm.tile([128, S], BF16, tag="attnT_a")
            attnT_b = perm.tile([64, S], BF16, tag="attnT_b")
            wot_a = perm.tile([128, D], BF16, tag="wot_a")
            wot_b = perm.tile([64, D], BF16, tag="wot_b")
            # window-1 raw inputs + q/k/v weights outlive phase 1: their
            # projections are deferred into the first attention block
            wq_sb = perm.tile([128, KT6, HD], BF16, tag="wq")
            wk_sb = perm.tile([128, KT6, HD], BF16, tag="wk")
            wv_sb = perm.tile([128, KT6, HD], BF16, tag="wv")
            k_raw1 = perm.tile([128, KT6, QB], BF16, tag="k_raw1")
            v_raw1 = perm.tile([128, KT6, QB], BF16, tag="v_raw1")
            q_raw1 = perm.tile([128, KT6, QB], BF16, tag="q_raw1")

            nc.vector.memset(v_sb[:], 1.0)
            nc.gpsimd.load_library(library_config.attn)

            mask_tiles = {}

            def load_masks(qq, groups):
                # one DMA per group of 4 k-tiles to amortize DGE overhead
                q0 = qq * QB
                tiles = mask_tiles.setdefault(qq, {})
                for g in groups:
                    m_g = mask_pool.tile([128, 4, QB], BF16, tag=f"mg{g}",
                                         name="m_g")
                    nc.sync.dma_start(
                        m_g[:], mt_r[:, g * 4:(g + 1) * 4, q0:q0 + QB])
                    for i in range(4):
                        tiles[g * 4 + i] = m_g[:, i, :]

            def project_dst(x_raw, w_sb, dst, mw, mt_i, wd, pool, tag,
                            copy_eng="act"):
                # dst[:mw, wd*QB:+QB] = (x_win @ w[:, mt_i*128:+mw]).T
                ps = pool.tile([128, QB], F32, tag=tag, name="ps_proj")
                for kt in range(KT6):
                    for half in range(QB // 512):
                        nc.tensor.matmul(
                            ps[:mw, half * 512:(half + 1) * 512],
                            w_sb[:, kt, mt_i * 128: mt_i * 128 + mw],
                            x_raw[:, kt, half * 512:(half + 1) * 512],
                            start=(kt == 0), stop=(kt == KT6 - 1))
                if copy_eng == "act":
                    nc.scalar.copy(dst[:mw, wd * QB:(wd + 1) * QB], ps[:mw, :])
                else:
                    nc.vector.tensor_copy(
                        dst[:mw, wd * QB:(wd + 1) * QB], ps[:mw, :])

            def project_v(x_raw, st, s_off, pool, tag, copy_eng="act"):
                # v_sb[:, st, :, 0:DK] = x_win[:, st_cols] @ w_v
                psv = pool.tile([128, QB] if tag == "ps" else [128, HD], F32,
                                tag=tag, name="psv")
                for kt in range(KT6):
                    nc.tensor.matmul(
                        psv[:, 0:HD],
                        x_raw[:, kt, s_off * 128:(s_off + 1) * 128],
                        wv_sb[:, kt, :],
                        start=(kt == 0), stop=(kt == KT6 - 1))
                if copy_eng == "act":
                    nc.scalar.copy(
                        v_sb[:, st, :, 0:DK],
                        psv[:, 0:HD].rearrange("p (h d) -> p h d", h=NH))
                else:
                    nc.vector.tensor_copy(
                        v_sb[:, st, :, 0:DK],
                        psv[:, 0:HD].rearrange("p (h d) -> p h d", h=NH))

            # ---- phase 1: window-0 loads + projections ----
            with (
                tc.tile_pool(name="raw", bufs=1) as raw_pool,
                tc.tile_pool(name="psum_proj", bufs=3, space="PSUM") as psum_proj,
                tc.tile_pool(name="psum_v", bufs=2, space="PSUM") as psum_v_pool,
            ):
                def p1_chunk(x_raw, w_sb, dst, mw, mt_i, half, eng):
                    # 512-col window-0 projection chunk: shortens the DMA ->
                    # project -> first-exp critical chain at startup
                    ps = psum_proj.tile([128, QB], F32, tag="ps_proj",
                                        name="ps_chunk")
                    for kt in range(KT6):
                        nc.tensor.matmul(
                            ps[:mw, 0:512],
                            w_sb[:, kt, mt_i * 128: mt_i * 128 + mw],
                            x_raw[:, kt, half * 512:(half + 1) * 512],
                            start=(kt == 0), stop=(kt == KT6 - 1))
                    if eng == "act":
                        nc.scalar.copy(
                            dst[:mw, half * 512:(half + 1) * 512],
                            ps[:mw, 0:512])
                    else:
                        nc.vector.tensor_copy(
                            dst[:mw, half * 512:(half + 1) * 512],
                            ps[:mw, 0:512])

                nc.sync.dma_start(
                    wk_sb[:], wkt_d.ap().rearrange("(o p) m -> p o m", p=128))
                k_raw0 = raw_pool.tile([128, KT6, QB], BF16, tag="k_raw0",
                                       name="k_raw0")
                nc.sync.dma_start(k_raw0[:, :, 0:512], kt_r[:, :, 0:512])
                p1_chunk(k_raw0, wk_sb, kT_a, 128, 0, 0, "act")
                p1_chunk(k_raw0, wk_sb, kT_b, 64, 1, 0, "dve")

                nc.sync.dma_start(
                    wq_sb[:], wqt_d.ap().rearrange("(o p) m -> p o m", p=128))
                q_raw0 = raw_pool.tile([128, KT6, QB], BF16, tag="q_raw0",
                                       name="q_raw0")
                # q projection chunks are emitted before the v one: the first
                # score matmuls need qT, while v_sb[st=0] is only read by the
                # first AV matmul, ~2 us later
                for half in range(2):
                    nc.sync.dma_start(
                        q_raw0[:, :, half * 512:(half + 1) * 512],
                        qt_r[:, :, half * 512:(half + 1) * 512])
                    p1_chunk(q_raw0, wq_sb, qT_a, 128, 0, half, "act")
                    p1_chunk(q_raw0, wq_sb, qT_b, 64, 1, half, "dve")
                nc.sync.dma_start(k_raw0[:, :, 512:QB], kt_r[:, :, 512:QB])
                p1_chunk(k_raw0, wk_sb, kT_a, 128, 0, 1, "act")
                p1_chunk(k_raw0, wk_sb, kT_b, 64, 1, 1, "dve")

                nc.sync.dma_start(
                    wv_sb[:], wvt_d.ap().rearrange("(o p) m -> p o m", p=128))
                v_raw0 = raw_pool.tile([128, KT6, QB], BF16, tag="v_raw0",
                                       name="v_raw0")
                for half in range(2):
                    nc.sync.dma_start(
                        v_raw0[:, :, half * 512:(half + 1) * 512],
                        vt_r[:, :, half * 512:(half + 1) * 512])
                    for st in range(half * 4, half * 4 + 4):
                        project_v(v_raw0, st, st, psum_v_pool, "psv", "dve")

                # window-1 loads + remaining masks stream in during attention
                load_masks(0, [0])
                nc.sync.dma_start(v_raw1[:], vt_r[:, :, QB:S])
                load_masks(0, [1])
                nc.sync.dma_start(k_raw1[:], kt_r[:, :, QB:S])
                load_masks(0, [2])
                nc.sync.dma_start(q_raw1[:], qt_r[:, :, QB:S])
                load_masks(0, [3])
                nc.sync.dma_start(wot_a[:], wot_d.ap()[0:128, :])
                nc.sync.dma_start(wot_b[:], wot_d.ap()[128:HD, :])

            # ---- phase 2: attention + interleaved projections/output ----
            with (
                tc.tile_pool(name="expp", bufs=6) as exp_pool,
                tc.tile_pool(name="expm", bufs=8) as expm_pool,
                tc.tile_pool(name="cpn", bufs=2) as cp_pool,
                tc.tile_pool(name="norm", bufs=2) as norm_pool,
                tc.tile_pool(name="psA", bufs=2, space="PSUM") as ps_pool,
                tc.tile_pool(name="avA", bufs=2, space="PSUM") as av_pool,
                tc.tile_pool(name="outp", bufs=3) as out_pool,
            ):
                head_src = [
                    (qT_a, kT_a, 0),    # h0: partitions 0-63
                    (qT_a, kT_a, 64),   # h1: partitions 64-127
                    (qT_b, kT_b, 0),    # h2
                ]

                def attend_scores(h, kt, q0, m_t):
                    # scores -> exp -> mask; the AV matmul is emitted one
                    # k-tile later so next tile's score MMs aren't stuck
                    # behind this tile's AV in the PE queue
                    qsrc, ksrc, p0 = head_src[h]
                    ps = ps_pool.tile([128, QB], F32, tag="ps", name="ps")
                    for half in range(QB // 512):
                        nc.tensor.matmul(
                            ps[:, half * 512:(half + 1) * 512],
                            ksrc[p0:p0 + DK, kt * 128:(kt + 1) * 128],
                            qsrc[p0:p0 + DK, q0 + half * 512:q0 + (half + 1) * 512],
                            start=True, stop=True)
                    ex = exp_pool.tile([128, QB], BF16, tag="e", name="ex")
                    nc.scalar.activation(ex[:], ps[:], AF.Exp, scale=0.125)
                    em = expm_pool.tile([128, QB], BF16, tag="em", name="em")
                    nc.vector.tensor_tensor(em[:], ex[:], m_t[:], ALU.mult)
                    return em

                def attend_av(h, kt, em, av):
                    for half in range(QB // 512):
                        nc.tensor.matmul(
                            av[0:DK + 1, half * 512:(half + 1) * 512],
                            v_sb[:, kt, h, :],
                            em[:, half * 512:(half + 1) * 512],
                            start=(kt == 0), stop=(kt == ST - 1))

                def normalize(h, q0, av, direct=False):
                    # free the PSUM tile fast: copy attn+denominator to SBUF,
                    # then normalize from the copy. When the ring buffer is
                    # not reused soon (h1 of each block, tail), read the PSUM
                    # directly and skip the copy.
                    if direct:
                        cp = av
                    else:
                        cp = cp_pool.tile([DK + 1, QB], BF16, tag="cp",
                                          name="cp")
                        nc.scalar.copy(cp[:], av[0:DK + 1, :])
                    rc = norm_pool.tile([1, QB], BF16, tag="rc", name="rc")
                    with nc.allow_low_precision(reason="softmax recip in bf16"):
                        nc.vector.reciprocal(rc[:], cp[DK:DK + 1, :])
                    bc = norm_pool.tile([64, QB], BF16, tag="bc", name="bc")
                    nc.gpsimd.partition_broadcast(bc[:], rc[:])
                    if h < 2:
                        dst = attnT_a[h * 64:(h + 1) * 64, q0:q0 + QB]
                    else:
                        dst = attnT_b[0:64, q0:q0 + QB]
                    nc.vector.tensor_tensor(dst, cp[0:DK, :], bc[:], ALU.mult)

                ob_tiles = {}

                def out_proj(st, copy_eng=None):
                    po = ps_pool.tile([128, QB], F32, tag="ps", name="po")
                    for (o, n) in [(0, 512), (512, 256)]:
                        nc.tensor.matmul(
                            po[:, o:o + n],
                            attnT_a[:, st * 128:(st + 1) * 128],
                            wot_a[:, o:o + n],
                            start=True, stop=False)
                        nc.tensor.matmul(
                            po[:, o:o + n],
                            attnT_b[0:64, st * 128:(st + 1) * 128],
                            wot_b[:, o:o + n],
                            start=False, stop=True)
                    # 4 s-tiles share one SBUF staging tile and one store DMA
                    g = st // 4
                    if g not in ob_tiles:
                        ob_tiles[g] = out_pool.tile([128, 4, D], BF16,
                                                    tag=f"ob{g % 2}", name="ob")
                    ob = ob_tiles[g]
                    if copy_eng == "act":
                        nc.scalar.copy(ob[:, st % 4, :], po[:, 0:D])
                    else:
                        nc.vector.tensor_copy(ob[:, st % 4, :], po[:, 0:D])
                    if st % 4 == 3:
                        nc.sync.dma_start(
                            out_r[:, g * 4:(g + 1) * 4, :], ob[:])

                def proj_chunk(x_raw, w_sb, dst, mw, mt_i, wd, half):
                    # half-window (512-col) deferred projection on the shared
                    # ring: short PSUM-ring hold
                    ps = ps_pool.tile([128, QB], F32, tag="ps", name="ps_proj")
                    for kt in range(KT6):
                        nc.tensor.matmul(
                            ps[:mw, 0:512],
                            w_sb[:, kt, mt_i * 128: mt_i * 128 + mw],
                            x_raw[:, kt, half * 512:(half + 1) * 512],
                            start=(kt == 0), stop=(kt == KT6 - 1))
                    nc.vector.tensor_copy(
                        dst[:mw, wd * QB + half * 512:wd * QB + (half + 1) * 512],
                        ps[:mw, 0:512])

                # deferred window-1 projections. Pass A (ACT-bound, ring
                # contended) only carries what pass A itself consumes: v
                # s-tiles 8-15 and the kT_a chunks needed from k-tile 8/12.
                deferred = {
                    4: [lambda: project_v(v_raw1, 8, 0, ps_pool, "ps", "dve"),
                        lambda: project_v(v_raw1, 9, 1, ps_pool, "ps", "dve")],
                    5: [lambda: project_v(v_raw1, 10, 2, ps_pool, "ps", "dve"),
                        lambda: project_v(v_raw1, 11, 3, ps_pool, "ps", "dve")],
                    6: [lambda: proj_chunk(k_raw1, wk_sb, kT_a, 128, 0, 1, 0)],
                    7: [lambda: project_v(v_raw1, 12, 4, ps_pool, "ps", "dve"),
                        lambda: project_v(v_raw1, 13, 5, ps_pool, "ps", "dve")],
                    8: [lambda: project_v(v_raw1, 14, 6, ps_pool, "ps", "dve"),
                        lambda: project_v(v_raw1, 15, 7, ps_pool, "ps", "dve")],
                    9: [lambda: proj_chunk(k_raw1, wk_sb, kT_a, 128, 0, 1, 1)],
                }
                # kT_b/qT_a/qT_b window 1 are only consumed by pass B / the
                # second block: project them during pass B where the PSUM
                # ring has slack
                deferred_b = {
                    0: [lambda: proj_chunk(k_raw1, wk_sb, kT_b, 64, 1, 1, 0)],
                    3: [lambda: proj_chunk(k_raw1, wk_sb, kT_b, 64, 1, 1, 1)],
                    6: [lambda: proj_chunk(q_raw1, wq_sb, qT_a, 128, 0, 1, 0)],
                    9: [lambda: proj_chunk(q_raw1, wq_sb, qT_a, 128, 0, 1, 1)],
                    12: [lambda: proj_chunk(q_raw1, wq_sb, qT_b, 64, 1, 1, 0)],
                    14: [lambda: proj_chunk(q_raw1, wq_sb, qT_b, 64, 1, 1, 1)],
                }

                for qq in range(NQ):
                    q0 = qq * QB
                    if qq not in mask_tiles:
                        load_masks(qq, range(ST // 4))
                    masks = [mask_tiles[qq][kt] for kt in range(ST)]

                    # pass A: heads 0 and 1; previous block\'s output
                    # projection rides along on the spare PE/PSUM slots
                    avs = [av_pool.tile([128, QB], F32, tag="av", name=f"av{h}")
                           for h in range(2)]
                    prev_st = list(range((qq - 1) * (QB // 128),
                                         qq * (QB // 128))) if qq > 0 else []
                    pending = []
                    for kt in range(ST):
                        ems = [attend_scores(h, kt, q0, masks[kt])
                               for h in range(2)]
                        pending.append((kt, ems))
                        if len(pending) > 2:
                            pkt, pems = pending.pop(0)
                            for h in range(2):
                                attend_av(h, pkt, pems[h], avs[h])
                        if kt % 2 == 1 and prev_st:
                            out_proj(prev_st.pop(0))
                        if qq == 0:
                            for fn in deferred.pop(kt, []):
                                fn()
                    for pkt, pems in pending:
                        for h in range(2):
                            attend_av(h, pkt, pems[h], avs[h])
                    # h1's ring buffer is next reused a full pass later:
                    # no need for the fast-free copy
                    normalize(0, q0, avs[0])
                    normalize(1, q0, avs[1], direct=True)

                    # pass B: head 2
                    av2 = av_pool.tile([128, QB], F32, tag="av", name="av2")
                    prev_em = None
                    for kt in range(ST):
                        em = attend_scores(2, kt, q0, masks[kt])
                        if prev_em is not None:
                            attend_av(2, kt - 1, prev_em, av2)
                        prev_em = em
                        if qq == 0:
                            for fn in deferred_b.pop(kt, []):
                                fn()
                    attend_av(2, ST - 1, prev_em, av2)
                    normalize(2, q0, av2, direct=(qq == NQ - 1))

                # tail: output projection for the last q-block; alternate the
                # staging copies across DVE and the now-idle ACT engine
                for i, st in enumerate(range((NQ - 1) * (QB // 128),
                                             NQ * (QB // 128))):
                    out_proj(st, copy_eng="act" if i % 2 else None)

    nc.compile()
    return nc


def make_in_maps(Q, K, V, mask, W_Q, W_K, W_V, W_O, n_cores=8):
    import ml_dtypes
    bf = ml_dtypes.bfloat16
    in_maps = []
    qt = [np.ascontiguousarray(Q[b].T).astype(bf) for b in range(2)]
    kt = [np.ascontiguousarray(K[b].T).astype(bf) for b in range(2)]
    vt = [np.ascontiguousarray(V[b].T).astype(bf) for b in range(2)]
    mt = [np.ascontiguousarray(mask[b, 0].T).astype(bf) for b in range(2)]
    wqt = W_Q.T.astype(bf)
    wkt = W_K.T.astype(bf)
    wvt = W_V.T.astype(bf)
    wot = W_O.T.astype(bf)
    for c in range(n_cores):
        b, g = divmod(c, 4)
        hs = slice(g * HD, (g + 1) * HD)
        in_maps.append({
            "qt": qt[b],
            "kt": kt[b],
            "vt": vt[b],
            "mt": mt[b],
            "wqt": np.ascontiguousarray(wqt[:, hs]),
            "wkt": np.ascontiguousarray(wkt[:, hs]),
            "wvt": np.ascontiguousarray(wvt[:, hs]),
            "wot": np.ascontiguousarray(wot[hs, :]),
        })
    return in_maps


def combine_outputs(partials):
    b0 = partials[0] + partials[1] + partials[2] + partials[3]
    b1 = partials[4] + partials[5] + partials[6] + partials[7]
    return np.stack([b0, b1])


_NC_CACHE = {}


def _get_nc(reps=1):
    key = ("nc", reps)
    if key not in _NC_CACHE:
        _NC_CACHE[key] = build_mha_nc(n_cores=8, reps=reps)
    return _NC_CACHE[key]


def kernel(Q, K, V, mask, W_Q, W_K, W_V, W_O, _reps=1, _trace=False):
    from concourse.bass_utils import run_bass_kernel_spmd
    nc = _get_nc(_reps)
    in_maps = make_in_maps(np.asarray(Q, np.float32), np.asarray(K, np.float32),
                           np.asarray(V, np.float32), np.asarray(mask),
                           np.asarray(W_Q, np.float32), np.asarray(W_K, np.float32),
                           np.asarray(W_V, np.float32), np.asarray(W_O, np.float32))
    res = run_bass_kernel_spmd(nc, in_maps, core_ids=list(range(8)),
                               trace=bool(_trace))
    out = combine_outputs([np.asarray(res.results[c]["out"], np.float32)
                           for c in range(8)])
    out = out.astype(np.float32)
    if _trace:
        kernel._last_trace = res
    return out
